# revision 1
# baseline (speedup 1.0000x reference)
"""Multi-head causal self-attention on 8 Trainium2 NeuronCores.

Sharding: heads are split 2-per-core (tensor parallel); every core computes
QKV projections for its 2 heads over the full batch, runs attention, then two
half-size 8-core AllToAll collectives (one per head, the first overlapped
with the second head's attention sweep) redistribute the per-head outputs to
sequence-sharded form for the row-parallel output projection. Matmuls run in
float32r (TF32-like).

Reference semantics (torch nn.Linear convention, y = x @ W.T):
  Q = x @ Wq.T ; K = x @ Wk.T ; V = x @ Wv.T           (split into 16 heads)
  scores = Q K^T / sqrt(64), causal-masked, softmax
  out = (softmax(scores) @ V, concat heads) @ Wo.T + bo
"""

import sys
from contextlib import ExitStack

sys.path.insert(0, "/opt/trn_rl_repo")

import numpy as np

import concourse.bass as bass  # noqa: E402
import concourse.mybir as mybir  # noqa: E402
from concourse import bacc  # noqa: E402
from concourse.bass_utils import run_bass_kernel_spmd  # noqa: E402
from concourse.masks import make_identity  # noqa: E402
from concourse.tile import TileContext  # noqa: E402

B = 2
S = 2048
D = 1024
H = 16
DK = 64
N_CORES = 8
HPC = H // N_CORES          # heads per core = 2
EL = HPC * DK               # local embedding slice = 128
P = 128                     # partitions
SBLK = 512                  # q-block (free dim of score matmuls)
NQ = S // SBLK              # q-blocks per batch = 4
NKT = S // P                # k-tiles per batch = 16
ND = D // P                 # d-tiles = 8
BS = B * S                  # flattened (batch, seq) = 4096
F32 = mybir.dt.float32
F32R = mybir.dt.float32r
BF16 = mybir.dt.bfloat16


def _classify_mask(mask: np.ndarray):
    """Classify each (q-block, k-tile) block of the [S, S] mask.

    Returns (blocks, patterns):
      blocks[j] = list of (t, pat_idx or None) k-tiles with any valid entry
      patterns  = float32 [n_pat, P, SBLK] multiplicative masks in [k, q]
                  layout for partially-valid blocks (deduplicated).
    """
    mask = np.asarray(mask).astype(bool)
    patterns = []
    pat_index = {}
    blocks = []
    for j in range(NQ):
        row = []
        sub_q = mask[j * SBLK:(j + 1) * SBLK]
        for t in range(NKT):
            sub = sub_q[:, t * P:(t + 1) * P]
            if not sub.any():
                continue
            if sub.all():
                row.append((t, None))
                continue
            patT = np.ascontiguousarray(sub.T).astype(np.float32)  # [k, q]
            key = patT.tobytes()
            if key not in pat_index:
                pat_index[key] = len(patterns)
                patterns.append(patT)
            row.append((t, pat_index[key]))
        assert row, f"q-block {j} has no valid keys; unsupported mask"
        blocks.append(row)
    if not patterns:
        patterns.append(np.ones((P, SBLK), np.float32))
    return blocks, np.stack(patterns)


def _build_program(blocks, n_pat):
    nc = bacc.Bacc("TRN2", target_bir_lowering=False, debug=False,
                   num_devices=N_CORES)

    # ---- I/O ----------------------------------------------------------
    # xT: [D, B*S] (x transposed, batches concatenated along columns)
    # weight inputs are pre-tiled on the host into SBUF layout [P, ND*cols]
    xT = nc.declare_dram_parameter("xT", [D, BS], F32R, isOutput=False)
    wqT = nc.declare_dram_parameter("wqT", [P, ND * EL], F32R,
                                    isOutput=False)
    wkT = nc.declare_dram_parameter("wkT", [P, ND * EL], F32R,
                                    isOutput=False)
    wvT = nc.declare_dram_parameter("wvT", [P, ND * EL], F32R,
                                    isOutput=False)
    woT = nc.declare_dram_parameter("woT", [P, ND * D], F32R,
                                    isOutput=False)
    bo = nc.declare_dram_parameter("bo", [1, D], F32, isOutput=False)
    mpat = nc.declare_dram_parameter("mpat", [P, n_pat * SBLK], F32R,
                                     isOutput=False)
    out = nc.declare_dram_parameter("out", [SBLK, D], F32, isOutput=True)

    # collective bounce buffers (internal DRAM), one pair per head sweep
    a2a_in = [nc.dram_tensor(f"a2a_in{h}", [N_CORES, DK, SBLK], F32R)
              for h in range(HPC)]
    a2a_out = [nc.dram_tensor(f"a2a_out{h}", [N_CORES, DK, SBLK], F32R)
               for h in range(HPC)]

    with TileContext(nc) as tc, ExitStack() as ctx:
        const = ctx.enter_context(tc.tile_pool(name="const", bufs=1))
        persist = ctx.enter_context(tc.tile_pool(name="persist", bufs=1))

        # constants
        ident = const.tile([P, P], F32)
        make_identity(nc, ident)
        bo_sb = const.tile([1, D], F32)
        nc.sync.dma_start(out=bo_sb[:], in_=bo[:])
        bo_bc = const.tile([P, D], F32)
        nc.gpsimd.partition_broadcast(bo_bc[:], bo_sb[:])
        w_sb = {}
        for name, t in (("wq", wqT), ("wk", wkT), ("wv", wvT)):
            w = const.tile([P, ND * EL], F32R, name=f"w_{name}",
                           tag=f"w_{name}")
            nc.sync.dma_start(out=w[:], in_=t[:])
            w_sb[name] = w
        mpat_sb = const.tile([P, n_pat * SBLK], F32R, tag="mpat")
        nc.sync.dma_start(out=mpat_sb[:], in_=mpat[:])
        zero_w = const.tile([P, DK + 1], F32R, tag="zero_w")
        nc.vector.memset(zero_w[:].bitcast(F32), 0.0)

        # persistent activations
        qT = persist.tile([P, BS], BF16, tag="qT")   # [EL, B*S]
        kT = persist.tile([P, BS], BF16, tag="kT")
        v_aug = [persist.tile([P, HPC * (DK + 1)], F32R,
                              name=f"vaug{i}", tag=f"vaug{i}")
                 for i in range(B * NKT)]
        stage = [persist.tile([P, SBLK], F32R, name=f"stg{i}",
                              tag=f"stg{i}")
                 for i in range(N_CORES)]

        HALF = BS // 2
        nsb = HALF // SBLK
        mpat3 = mpat_sb[:].rearrange("p (n q) -> p n q", n=n_pat)

        with ExitStack() as pmain:
            probs_pool = pmain.enter_context(tc.tile_pool(name="probs",
                                                          bufs=5))
            small = pmain.enter_context(tc.tile_pool(name="small", bufs=4))
            px = ExitStack()
            xpool = px.enter_context(tc.tile_pool(name="xT", bufs=ND))
            vt_pool = px.enter_context(tc.tile_pool(name="vt", bufs=1))
            # PSUM budget: ps_qk 2 + ps_sc 2*2 + ps_out 2 = 8 banks.
            # ps_out's tag also serves the V-transpose psums.
            ps_qk = pmain.enter_context(
                tc.tile_pool(name="ps_qk", bufs=2, space="PSUM"))
            ps_sc = pmain.enter_context(
                tc.tile_pool(name="ps_sc", bufs=2, space="PSUM"))
            ps_out = pmain.enter_context(
                tc.tile_pool(name="ps_out", bufs=2, space="PSUM"))

            # ---- emission helpers -----------------------------------
            def load_half(half):
                xt = []
                for k in range(ND):
                    t = xpool.tile([P, HALF], F32R, tag="xt")
                    nc.sync.dma_start(
                        out=t[:],
                        in_=xT[k * P:(k + 1) * P,
                               half * HALF:(half + 1) * HALF])
                    xt.append(t)
                return xt

            def proj_group(half, xt, vt, name, sb):
                dest = {"wq": qT, "wk": kT, "wv": vt}[name]
                ps = ps_qk.tile([P, SBLK], F32, tag="ps_qk",
                                name=f"ps_{name}_{half}_{sb}")
                for k in range(ND):
                    nc.tensor.matmul(
                        ps[:], w_sb[name][:, bass.ts(k, EL)],
                        xt[k][:, bass.ts(sb, SBLK)],
                        start=(k == 0), stop=(k == ND - 1))
                if name == "wv":
                    nc.vector.tensor_copy(vt[:, bass.ts(sb, SBLK)], ps[:])
                else:
                    osl = bass.ds(half * HALF + sb * SBLK, SBLK)
                    nc.vector.tensor_copy(dest[:, osl], ps[:])

            def transpose_v(half, vt, i):
                tile_idx = half * (HALF // P) + i
                ps = ps_out.tile([P, SBLK], F32, tag="ps_out",
                                 name=f"tr_{half}_{i}")
                nc.tensor.matmul(ps[:, 0:P],
                                 vt[:, bass.ts(i, P)].bitcast(F32),
                                 ident[:], is_transpose=True)
                va = v_aug[tile_idx]
                va3 = va[:].rearrange("p (h e) -> p h e", h=HPC)
                nc.vector.tensor_copy(
                    va3[:, :, 0:DK],
                    ps[:, 0:P].rearrange("p (h e) -> p h e", h=HPC))
                nc.vector.memset(va3[:, :, DK:DK + 1].bitcast(F32), 1.0)

            def emit_scores(it, pair):
                h, b, j = it["h"], it["b"], it["j"]
                hsl = bass.ds(h * DK, DK)
                q0 = b * S + j * SBLK
                w = len(pair) * SBLK
                ps = ps_sc.tile([P, 2 * SBLK], F32, tag="ps_sc")
                for i, (t, _pat) in enumerate(pair):
                    nc.tensor.matmul(
                        ps[:, bass.ts(i, SBLK)],
                        kT[hsl, bass.ds(b * S + t * P, P)],
                        qT[hsl, bass.ds(q0, SBLK)],
                        start=True, stop=True)
                pr = probs_pool.tile([P, 2 * SBLK], F32R, tag="probs")
                nc.scalar.activation(pr[:, 0:w], ps[:, 0:w],
                                     mybir.ActivationFunctionType.Exp)
                pats = [pat for _t, pat in pair]
                if (len(pats) == 2 and pats[0] is not None
                        and pats[1] == pats[0] + 1):
                    nc.vector.tensor_mul(
                        pr[:, 0:2 * SBLK], pr[:, 0:2 * SBLK],
                        mpat_sb[:, pats[0] * SBLK:(pats[0] + 2) * SBLK])
                else:
                    for i, (t, pat) in enumerate(pair):
                        if pat is not None:
                            nc.vector.tensor_mul(pr[:, bass.ts(i, SBLK)],
                                                 pr[:, bass.ts(i, SBLK)],
                                                 mpat3[:, pat])
                return pr

            def emit_attnv(it, pair, pr, filler):
                h, b = it["h"], it["b"]
                if it["po"] is None:
                    it["po"] = ps_out.tile([P, SBLK], F32, tag="ps_out",
                                           name=f"po_{h}_{b}_{it['j']}")
                po = it["po"]
                for i, (t, _pat) in enumerate(pair):
                    it["n_mm"] += 1
                    nc.tensor.matmul(
                        po[0:DK + 1, :],
                        v_aug[b * NKT + t][:, bass.ds(h * (DK + 1),
                                                      DK + 1)],
                        pr[:, bass.ts(i, SBLK)],
                        start=(it["n_mm"] == 1),
                        stop=(it["n_mm"] == it["total"]))
                    if filler and it["n_mm"] < it["total"]:
                        # zero-contribution matmul keeps the PE array
                        # streaming so the HAM clock gate stays at 2.4 GHz
                        it["n_mm"] += 1
                        nc.tensor.matmul(
                            po[0:DK + 1, :], zero_w[:],
                            pr[:, bass.ts(i, SBLK)],
                            start=False,
                            stop=(it["n_mm"] == it["total"]))
                if it["n_mm"] >= it["total"]:
                    emit_norm(it)

            def emit_norm(it):
                h, b, j = it["h"], it["b"], it["j"]
                po = it["po"]
                un = small.tile([DK, SBLK], F32, tag="un")
                nc.vector.tensor_copy(un[:], po[0:DK, :])
                sumr = small.tile([1, SBLK], F32, tag="sumr")
                nc.vector.tensor_copy(sumr[:], po[DK:DK + 1, :])
                rb = small.tile([DK, SBLK], F32, tag="rb")
                nc.gpsimd.partition_broadcast(rb[:], sumr[:])
                nc.vector.reciprocal_approx_fast(out=rb[:], in_=rb[:])
                nc.vector.tensor_mul(
                    stage[b * NQ + j][h * DK:(h + 1) * DK, :],
                    un[:], rb[:])
                nc.sync.dma_start(
                    out=a2a_in[h][b * NQ + j],
                    in_=stage[b * NQ + j][h * DK:(h + 1) * DK, :])

            from collections import deque
            pend = deque()
            LAG = 4

            def attn_units(h, b, filler):
                """One generator step per pair emission for sweep (h, b)."""
                for j in range(NQ):
                    tiles = blocks[j]
                    nfill = len(tiles) - 1 if filler else 0
                    it = {"h": h, "b": b, "j": j, "po": None, "n_mm": 0,
                          "total": len(tiles) + nfill}
                    for c0 in range(0, len(tiles), 2):
                        pair = tiles[c0:c0 + 2]
                        pr = emit_scores(it, pair)
                        pend.append((it, pair, pr, filler))
                        if len(pend) > LAG:
                            emit_attnv(*pend.popleft())
                        yield

            def drain():
                while pend:
                    emit_attnv(*pend.popleft())

            # ---- master schedule ------------------------------------
            # warm the PE clock gate during the initial DMA wait so the
            # first projection groups run at 2.4 GHz
            wps = ps_qk.tile([P, SBLK], F32, tag="ps_qk", name="warm_ps")
            for r in range(24):
                nc.tensor.matmul(wps[0:DK + 1, :], zero_w[:],
                                 mpat_sb[:, 0:SBLK], start=True, stop=True)
            xt0 = load_half(0)
            vt0 = vt_pool.tile([P, HALF], F32R, tag="vt", name="vt0")
            for name in ("wq", "wk", "wv"):
                for sb in range(nsb):
                    proj_group(0, xt0, vt0, name, sb)
            for i in range(HALF // P):
                transpose_v(0, vt0, i)

            # interleave half-1 projections with (h0, b0) attention
            xt1 = load_half(1)
            vt1 = vt_pool.tile([P, HALF], F32R, tag="vt", name="vt1")
            attn0 = attn_units(0, 0, filler=False)
            for name in ("wq", "wk", "wv"):
                for sb in range(nsb):
                    proj_group(1, xt1, vt1, name, sb)
                    next(attn0, None)
                    next(attn0, None)
            for _ in attn0:
                pass
            for i in range(HALF // P):
                transpose_v(1, vt1, i)
            px.close()

            for _ in attn_units(0, 1, filler=True):
                pass
            drain()
            nc.gpsimd.collective_compute(
                "AllToAll", mybir.AluOpType.bypass,
                replica_groups=[list(range(N_CORES))],
                ins=[a2a_in[0][:]], outs=[a2a_out[0][:]])

            for b in range(B):
                for _ in attn_units(1, b, filler=True):
                    pass
            drain()

            # phase-3 SBUF loads issued before the second collective so
            # the DMAs overlap the attention tail / collective
            with ExitStack() as p3:
                cat_pool = p3.enter_context(tc.tile_pool(name="cat",
                                                         bufs=1))
                osb_pool = p3.enter_context(tc.tile_pool(name="osb",
                                                         bufs=3))
                woT_sb = cat_pool.tile([P, ND * D], F32R, tag="woT")
                nc.sync.dma_start(out=woT_sb[:], in_=woT[:])
                cat = []
                for i in range(N_CORES):
                    t = cat_pool.tile([P, SBLK], F32R, name=f"cat{i}",
                                      tag=f"cat{i}")
                    nc.sync.dma_start(out=t[0:DK, :], in_=a2a_out[0][i])
                    cat.append(t)

                nc.gpsimd.collective_compute(
                    "AllToAll", mybir.AluOpType.bypass,
                    replica_groups=[list(range(N_CORES))],
                    ins=[a2a_in[1][:]], outs=[a2a_out[1][:]])
                for i in range(N_CORES):
                    nc.sync.dma_start(out=cat[i][DK:2 * DK, :],
                                      in_=a2a_out[1][i])

                for st in range(SBLK // P):
                    pss = [ps_qk.tile([P, SBLK], F32, tag="ps_qk",
                                      name=f"ps_f_{st}_{eb}")
                           for eb in range(D // SBLK)]
                    for k in range(ND):
                        for eb in range(D // SBLK):
                            nc.tensor.matmul(
                                pss[eb][:], cat[k][:, bass.ts(st, P)],
                                woT_sb[:, bass.ds(k * D + eb * SBLK,
                                                  SBLK)],
                                start=(k == 0), stop=(k == ND - 1))
                    for eb in range(D // SBLK):
                        ot = osb_pool.tile([P, SBLK], F32, tag="osb")
                        nc.vector.tensor_add(ot[:], pss[eb][:],
                                             bo_bc[:, bass.ts(eb, SBLK)])
                        nc.sync.dma_start(
                            out=out[st * P:(st + 1) * P,
                                    eb * SBLK:(eb + 1) * SBLK],
                            in_=ot[:])

    nc.compile()
    return nc


def _sbuf_tiled(wT):
    # [D, E] -> [P, ND*E]: row p holds d-tiles k at columns [k*E, (k+1)*E)
    dd, e = wT.shape
    return np.ascontiguousarray(
        wT.reshape(dd // P, P, e).transpose(1, 0, 2).reshape(P, -1))


def _prepare_inputs(x, Wq, Wk, Wv, Wo, bo, patterns):
    x = np.asarray(x, np.float32)
    xT = np.ascontiguousarray(
        np.concatenate([x[b].T for b in range(B)], axis=1))
    woT = _sbuf_tiled(np.ascontiguousarray(np.asarray(Wo, np.float32).T))
    bo2 = np.asarray(bo, np.float32).reshape(1, D)
    scale = np.float32(1.0 / np.sqrt(DK))
    n_pat = patterns.shape[0]
    mpat2 = np.ascontiguousarray(
        patterns.transpose(1, 0, 2).reshape(P, n_pat * SBLK))
    in_maps = []
    for c in range(N_CORES):
        cols = slice(c * EL, (c + 1) * EL)
        in_maps.append({
            "xT": xT,
            "wqT": _sbuf_tiled(np.asarray(Wq, np.float32).T[:, cols] * scale),
            "wkT": _sbuf_tiled(np.asarray(Wk, np.float32).T[:, cols]),
            "wvT": _sbuf_tiled(np.asarray(Wv, np.float32).T[:, cols]),
            "woT": woT,
            "bo": bo2,
            "mpat": mpat2,
        })
    return in_maps


def _run(inputs, trace=False):
    blocks, patterns = _classify_mask(inputs["mask"])
    nc = _build_program(blocks, patterns.shape[0])
    in_maps = _prepare_inputs(inputs["x"], inputs["Wq"], inputs["Wk"],
                              inputs["Wv"], inputs["Wo"], inputs["bo"],
                              patterns)
    res = run_bass_kernel_spmd(nc, in_maps, list(range(N_CORES)),
                               trace=trace)
    full = np.empty((B, S, D), np.float32)
    for c in range(N_CORES):
        b, j = divmod(c, NQ)
        full[b, j * SBLK:(j + 1) * SBLK, :] = res.results[c]["out"]
    return full, res


def kernel(**inputs) -> np.ndarray:
    out, _ = _run(inputs, trace=False)
    return out



# revision 16
# speedup vs baseline: 1.2373x; 1.2373x over previous
"""Multi-head causal self-attention on 8 Trainium2 NeuronCores.

Sharding: heads are split 2-per-core (tensor parallel); every core computes
QKV projections for its 2 heads over the full batch (bf16 activations and
weights), runs attention, then two half-size 8-core AllToAll collectives
(bf16, one per head, the first overlapped with the second head's attention
sweep) redistribute per-head outputs to sequence-sharded form for the
row-parallel output projection.

Schedule: x streams in per 512-token column block; batch-0 QKV projection
runs under the DMA, batch-1 projection interleaves with batch-0 attention
sweeps, and the h0 half of the output projection runs under the second
AllToAll so the PE never waits on the collective tail.

Reference semantics (torch nn.Linear convention, y = x @ W.T):
  Q = x @ Wq.T ; K = x @ Wk.T ; V = x @ Wv.T           (split into 16 heads)
  scores = Q K^T / sqrt(64), causal-masked, softmax
  out = (softmax(scores) @ V, concat heads) @ Wo.T + bo
"""

import sys
from collections import deque
from contextlib import ExitStack

sys.path.insert(0, "/opt/trn_rl_repo")

import numpy as np
import ml_dtypes

import concourse.bass as bass  # noqa: E402
import concourse.mybir as mybir  # noqa: E402
from concourse import bacc  # noqa: E402
from concourse.bass_utils import run_bass_kernel_spmd  # noqa: E402
from concourse.masks import make_identity  # noqa: E402
from concourse.tile import TileContext  # noqa: E402

B = 2
S = 2048
D = 1024
H = 16
DK = 64
N_CORES = 8
HPC = H // N_CORES          # heads per core = 2
EL = HPC * DK               # local embedding slice = 128
P = 128                     # partitions
SBLK = 512                  # q-block (free dim of score matmuls)
NQ = S // SBLK              # q-blocks per batch = 4
NKT = S // P                # k-tiles per batch = 16
ND = D // P                 # d-tiles = 8
BS = B * S                  # flattened (batch, seq) = 4096
NCB = BS // SBLK            # 512-token column blocks = 8
F32 = mybir.dt.float32
F32R = mybir.dt.float32r
BF16 = mybir.dt.bfloat16
BF16NP = ml_dtypes.bfloat16


def _classify_mask(mask: np.ndarray):
    """Classify each (q-block, k-tile) block of the [S, S] mask.

    Returns (blocks, patterns):
      blocks[j] = list of (t, pat_idx or None) k-tiles with any valid entry
      patterns  = float32 [n_pat, P, SBLK] multiplicative masks in [k, q]
                  layout for partially-valid blocks (deduplicated).
    """
    mask = np.asarray(mask).astype(bool)
    patterns = []
    pat_index = {}
    blocks = []
    for j in range(NQ):
        row = []
        sub_q = mask[j * SBLK:(j + 1) * SBLK]
        for t in range(NKT):
            sub = sub_q[:, t * P:(t + 1) * P]
            if not sub.any():
                continue
            if sub.all():
                row.append((t, None))
                continue
            patT = np.ascontiguousarray(sub.T).astype(np.float32)  # [k, q]
            key = patT.tobytes()
            if key not in pat_index:
                pat_index[key] = len(patterns)
                patterns.append(patT)
            row.append((t, pat_index[key]))
        assert row, f"q-block {j} has no valid keys; unsupported mask"
        blocks.append(row)
    if not patterns:
        patterns.append(np.ones((P, SBLK), np.float32))
    return blocks, np.stack(patterns)


DEBUG_DUMP = False


def _build_program(blocks, n_pat):
    nc = bacc.Bacc("TRN2", target_bir_lowering=False, debug=False,
                   num_devices=N_CORES)

    # ---- I/O ----------------------------------------------------------
    # xT: [D, B*S] bf16 (x transposed, batches concatenated along columns)
    # weight inputs are pre-tiled on the host into SBUF layout [P, ND*cols]
    xT = nc.declare_dram_parameter("xT", [D, BS], BF16, isOutput=False)
    wqT = nc.declare_dram_parameter("wqT", [P, ND * EL], BF16,
                                    isOutput=False)
    wkT = nc.declare_dram_parameter("wkT", [P, ND * EL], BF16,
                                    isOutput=False)
    wvT = nc.declare_dram_parameter("wvT", [P, ND * EL], BF16,
                                    isOutput=False)
    woT = nc.declare_dram_parameter("woT", [P, ND * D], BF16,
                                    isOutput=False)
    bo = nc.declare_dram_parameter("bo", [1, D], F32, isOutput=False)
    mpat = nc.declare_dram_parameter("mpat", [P, n_pat * SBLK], BF16,
                                     isOutput=False)
    out = nc.declare_dram_parameter("out", [SBLK, D], F32, isOutput=True)

    # collective bounce buffers (internal DRAM), one pair per head
    a2a_in = [nc.dram_tensor(f"a2a_in{h}", [N_CORES, DK, SBLK], BF16)
              for h in range(HPC)]
    a2a_out = [nc.dram_tensor(f"a2a_out{h}", [N_CORES, DK, SBLK], BF16)
               for h in range(HPC)]
    dbg = {}
    if DEBUG_DUMP:
        for nm, shp in (("dbg_q", [P, BS]), ("dbg_k", [P, BS]),
                        ("dbg_va", [P, B * (S // P) * HPC * (DK + 1)]),
                        ("dbg_stage", [P, N_CORES * SBLK])):
            dbg[nm] = nc.declare_dram_parameter(nm, shp, BF16, isOutput=True)

    with TileContext(nc) as tc, ExitStack() as ctx:
        const = ctx.enter_context(tc.tile_pool(name="const", bufs=1))
        persist = ctx.enter_context(tc.tile_pool(name="persist", bufs=1))

        # constants
        ident = const.tile([P, P], F32)
        make_identity(nc, ident)
        bo_sb = const.tile([1, D], F32)
        nc.sync.dma_start(out=bo_sb[:], in_=bo[:])
        bo_bc = const.tile([P, D], F32)
        nc.gpsimd.partition_broadcast(bo_bc[:], bo_sb[:])
        w_sb = {}
        for name, t in (("wq", wqT), ("wk", wkT), ("wv", wvT)):
            w = const.tile([P, ND * EL], BF16, name=f"w_{name}",
                           tag=f"w_{name}")
            nc.sync.dma_start(out=w[:], in_=t[:])
            w_sb[name] = w
        # mpat / woT DMAs are emitted later (first use is phases B / D) so
        # their transfers don't delay the x stream at startup
        mpat_sb = const.tile([P, n_pat * SBLK], BF16, tag="mpat")
        zero_w = const.tile([P, DK + 1], BF16, tag="zero_w")
        nc.vector.memset(zero_w[:], 0.0)
        wrm = const.tile([P, SBLK], BF16, tag="wrm")
        nc.vector.memset(wrm[:], 0.0)
        woT_sb = const.tile([P, ND * D], BF16, tag="woT")

        # persistent activations
        qT = persist.tile([P, BS], BF16, tag="qT")   # [EL, B*S]
        kT = persist.tile([P, BS], BF16, tag="kT")
        # va: per 128-token tile t, cols [t*130, t*130+130) hold
        # [h0 v 64 | ones | h1 v 64 | ones]  (keys on partitions)
        VA_W = HPC * (DK + 1)
        va = persist.tile([P, B * NKT * VA_W], BF16, tag="va")
        va4 = va[:].rearrange("p (t h e) -> p t h e", t=B * NKT, h=HPC)
        nc.vector.memset(va4[:, :, :, DK:DK + 1], 1.0)
        # stage: rows [h*64,(h+1)*64) x cols [c*512,(c+1)*512) = normalized
        # head-h output for block c = b*NQ+j
        stage = persist.tile([P, N_CORES * SBLK], BF16, tag="stage")

        mpat3 = mpat_sb[:].rearrange("p (n q) -> p n q", n=n_pat)

        with ExitStack() as pmain:
            probs_pool = pmain.enter_context(tc.tile_pool(name="probs",
                                                          bufs=6))
            small = pmain.enter_context(tc.tile_pool(name="small", bufs=4))
            px = ExitStack()
            xpool = px.enter_context(tc.tile_pool(name="xT", bufs=2))
            vt_pool = px.enter_context(tc.tile_pool(name="vt", bufs=2))
            # PSUM budget: psA 2 + po 2 + ps_sc 2*2 = 8 banks
            psA = pmain.enter_context(
                tc.tile_pool(name="psA", bufs=2, space="PSUM"))
            po_pool = pmain.enter_context(
                tc.tile_pool(name="po", bufs=2, space="PSUM"))
            ps_sc = pmain.enter_context(
                tc.tile_pool(name="ps_sc", bufs=2, space="PSUM"))

            ones_w = const.tile([1, DK], F32R, tag="ones_w")
            nc.vector.memset(ones_w[:].bitcast(F32), 1.0)

            # ---- emission helpers -----------------------------------
            def proj_block(cb):
                """QKV projection for one 512-token column block."""
                xt = xpool.tile([P, ND * SBLK], BF16, tag="xt",
                                name=f"xt{cb}")
                # xt[p, k*512+c] = xT[k*128+p, cb*512+c]
                nc.sync.dma_start(
                    out=xt[:].rearrange("p (k c) -> p k c", k=ND),
                    in_=xT[:, cb * SBLK:(cb + 1) * SBLK]
                    .rearrange("(k p) c -> p k c", p=P))
                osl = bass.ds(cb * SBLK, SBLK)
                for name in ("wq", "wk"):
                    ps = psA.tile([P, SBLK], F32, tag="psA",
                                  name=f"ps_{name}_{cb}")
                    for k in range(ND):
                        nc.tensor.matmul(
                            ps[:], w_sb[name][:, bass.ts(k, EL)],
                            xt[:, bass.ts(k, SBLK)],
                            start=(k == 0), stop=(k == ND - 1))
                    dest = qT if name == "wq" else kT
                    nc.vector.tensor_copy(dest[:, osl], ps[:])
                ps = psA.tile([P, SBLK], F32, tag="psA", name=f"ps_wv_{cb}")
                for k in range(ND):
                    nc.tensor.matmul(
                        ps[:], w_sb["wv"][:, bass.ts(k, EL)],
                        xt[:, bass.ts(k, SBLK)],
                        start=(k == 0), stop=(k == ND - 1))
                vt = vt_pool.tile([P, SBLK], F32R, tag="vt",
                                  name=f"vt{cb}")
                nc.vector.tensor_copy(vt[:], ps[:])
                # transpose V into va (keys on partitions)
                for i in range(SBLK // P):
                    tile_idx = cb * (SBLK // P) + i
                    pst = psA.tile([P, SBLK], F32, tag="psA",
                                   name=f"tr_{cb}_{i}")
                    nc.tensor.matmul(pst[:, 0:P],
                                     vt[:, bass.ts(i, P)].bitcast(F32),
                                     ident[:], is_transpose=True)
                    nc.vector.tensor_copy(
                        va4[:, tile_idx, :, 0:DK],
                        pst[:, 0:P].rearrange("p (h e) -> p h e", h=HPC))

            def emit_scores(it, pair):
                h, b, j = it["h"], it["b"], it["j"]
                hsl = bass.ds(h * DK, DK)
                q0 = b * S + j * SBLK
                w = len(pair) * SBLK
                ps = ps_sc.tile([P, 2 * SBLK], F32, tag="ps_sc")
                for i, (t, _pat) in enumerate(pair):
                    nc.tensor.matmul(
                        ps[:, bass.ts(i, SBLK)],
                        kT[hsl, bass.ds(b * S + t * P, P)],
                        qT[hsl, bass.ds(q0, SBLK)],
                        start=True, stop=True)
                pr = probs_pool.tile([P, 2 * SBLK], BF16, tag="probs")
                nc.scalar.activation(pr[:, 0:w], ps[:, 0:w],
                                     mybir.ActivationFunctionType.Exp)
                pats = [pat for _t, pat in pair]
                if (len(pats) == 2 and pats[0] is not None
                        and pats[1] == pats[0] + 1):
                    nc.vector.tensor_mul(
                        pr[:, 0:2 * SBLK], pr[:, 0:2 * SBLK],
                        mpat_sb[:, pats[0] * SBLK:(pats[0] + 2) * SBLK])
                else:
                    for i, (t, pat) in enumerate(pair):
                        if pat is not None:
                            nc.vector.tensor_mul(pr[:, bass.ts(i, SBLK)],
                                                 pr[:, bass.ts(i, SBLK)],
                                                 mpat3[:, pat])
                return pr

            def emit_attnv(it, pair, pr):
                h, b = it["h"], it["b"]
                if it["po"] is None:
                    it["po"] = po_pool.tile([P, SBLK], F32, tag="po",
                                            name=f"po_{h}_{b}_{it['j']}")
                po = it["po"]
                for i, (t, _pat) in enumerate(pair):
                    it["n_mm"] += 1
                    nc.tensor.matmul(
                        po[0:DK + 1, :],
                        va[:, bass.ds((b * NKT + t) * VA_W + h * (DK + 1),
                                      DK + 1)],
                        pr[:, bass.ts(i, SBLK)],
                        start=(it["n_mm"] == 1),
                        stop=(it["n_mm"] == it["total"]))
                if it["n_mm"] >= it["total"]:
                    emit_norm(it)

            def emit_norm(it):
                h, b, j = it["h"], it["b"], it["j"]
                po = it["po"]
                sumr = small.tile([1, SBLK], F32, tag="sumr")
                nc.vector.tensor_copy(sumr[:], po[DK:DK + 1, :])
                rc = small.tile([1, SBLK], F32, tag="rc")
                nc.vector.reciprocal_approx_fast(out=rc[:], in_=sumr[:])
                rb = small.tile([DK, SBLK], F32, tag="rb")
                nc.gpsimd.partition_broadcast(rb[:], rc[:])
                nc.vector.tensor_mul(
                    stage[h * DK:(h + 1) * DK,
                          bass.ts(b * NQ + j, SBLK)],
                    po[0:DK, :], rb[:])

            pend = deque()
            LAG = 4

            def attn_units(h, b):
                """One generator step per pair emission for sweep (h, b)."""
                for j in range(NQ):
                    tiles = blocks[j]
                    it = {"h": h, "b": b, "j": j, "po": None, "n_mm": 0,
                          "total": len(tiles)}
                    for c0 in range(0, len(tiles), 2):
                        pair = tiles[c0:c0 + 2]
                        pr = emit_scores(it, pair)
                        pend.append((it, pair, pr))
                        if len(pend) > LAG:
                            emit_attnv(*pend.popleft())
                        yield

            def drain():
                while pend:
                    emit_attnv(*pend.popleft())

            def stage_dma(h):
                # one trigger: stage rows of head h -> a2a_in[h][8,64,512]
                nc.sync.dma_start(
                    out=a2a_in[h][:].rearrange("c p q -> p c q"),
                    in_=stage[h * DK:(h + 1) * DK, :]
                    .rearrange("p (c q) -> p c q", c=N_CORES))

            # ---- master schedule ------------------------------------
            # warm the PE clock gate while the first DMAs land
            wps = psA.tile([P, SBLK], F32, tag="psA", name="warm_ps")
            for r in range(10):
                nc.tensor.matmul(wps[0:DK + 1, :], zero_w[:], wrm[:],
                                 start=True, stop=True)

            # phase A: batch-0 projection, pipelined with x DMA
            for cb in range(NCB // 2):
                proj_block(cb)
                if cb == 0:
                    nc.sync.dma_start(out=mpat_sb[:], in_=mpat[:])

            # phase B: batch-1 projection interleaved with batch-0 attention
            gens = [attn_units(0, 0), attn_units(1, 0)]

            def next_unit():
                while gens:
                    try:
                        next(gens[0])
                        return True
                    except StopIteration:
                        gens.pop(0)
                return False

            for cb in range(NCB // 2, NCB):
                proj_block(cb)
                for _ in range(10):
                    next_unit()
            while next_unit():
                pass
            drain()
            px.close()

            # phase C: batch-1 attention; A2A0 overlaps the (h1,b1) sweep
            for _ in attn_units(0, 1):
                pass
            drain()
            stage_dma(0)
            nc.gpsimd.collective_compute(
                "AllToAll", mybir.AluOpType.bypass,
                replica_groups=[list(range(N_CORES))],
                ins=[a2a_in[0][:]], outs=[a2a_out[0][:]])
            nc.sync.dma_start(out=woT_sb[:], in_=woT[:])

            for _ in attn_units(1, 1):
                pass
            drain()
            stage_dma(1)
            if DEBUG_DUMP:
                nc.sync.dma_start(out=dbg["dbg_q"][:], in_=qT[:])
                nc.sync.dma_start(out=dbg["dbg_k"][:], in_=kT[:])
                nc.sync.dma_start(out=dbg["dbg_va"][:], in_=va[:])
                nc.sync.dma_start(out=dbg["dbg_stage"][:], in_=stage[:])
            nc.gpsimd.collective_compute(
                "AllToAll", mybir.AluOpType.bypass,
                replica_groups=[list(range(N_CORES))],
                ins=[a2a_in[1][:]], outs=[a2a_out[1][:]])
            pmain.close()

            # phase D: output projection; h0 half runs under A2A1
            cat = [None, None]
            with ExitStack() as p3:
                cat_pool = p3.enter_context(tc.tile_pool(name="cat",
                                                         bufs=1))
                osb_pool = p3.enter_context(tc.tile_pool(name="osb",
                                                         bufs=3))

                def cat_dma(h):
                    cat[h] = cat_pool.tile([P, NQ * SBLK], BF16,
                                           name=f"cat{h}", tag=f"cat{h}")
                    src = a2a_out[h][:].rearrange("(k two) p q -> two p k q",
                                                  two=2)
                    for two in range(2):
                        nc.sync.dma_start(
                            out=cat[h][two * DK:(two + 1) * DK, :]
                            .rearrange("p (k q) -> p k q", k=NQ),
                            in_=src[two])

                cat_dma(0)
                with tc.tile_pool(name="ps_o", bufs=8,
                                  space="PSUM") as ps_o:
                    pss = {}
                    NEB = D // SBLK
                    for st in range(SBLK // P):
                        for eb in range(NEB):
                            pss[st, eb] = ps_o.tile(
                                [P, SBLK], F32, tag="ps_o",
                                name=f"ps_f_{st}_{eb}")
                    # h0 contraction tiles run while A2A1 is on the wire
                    for half in range(2):
                        if half == 1:
                            cat_dma(1)
                        for st in range(SBLK // P):
                            for eb in range(NEB):
                                for k in range(NQ):
                                    kk = half * NQ + k
                                    nc.tensor.matmul(
                                        pss[st, eb][:],
                                        cat[half][:, bass.ds(
                                            k * SBLK + st * P, P)],
                                        woT_sb[:, bass.ds(kk * D + eb * SBLK,
                                                          SBLK)],
                                        start=(kk == 0),
                                        stop=(kk == 2 * NQ - 1))
                    for st in range(SBLK // P):
                        for eb in range(NEB):
                            ot = osb_pool.tile([P, SBLK], F32, tag="osb")
                            nc.vector.tensor_add(
                                ot[:], pss[st, eb][:],
                                bo_bc[:, bass.ts(eb, SBLK)])
                            nc.sync.dma_start(
                                out=out[st * P:(st + 1) * P,
                                        eb * SBLK:(eb + 1) * SBLK],
                                in_=ot[:])

    nc.compile()
    return nc


def _sbuf_tiled(wT):
    # [D, E] -> [P, ND*E]: row p holds d-tiles k at columns [k*E, (k+1)*E)
    dd, e = wT.shape
    return np.ascontiguousarray(
        wT.reshape(dd // P, P, e).transpose(1, 0, 2).reshape(P, -1))


def _prepare_inputs(x, Wq, Wk, Wv, Wo, bo, patterns):
    x = np.asarray(x, np.float32)
    xT = np.ascontiguousarray(
        np.concatenate([x[b].T for b in range(B)], axis=1)).astype(BF16NP)
    # output projection: concat dims permuted into head-pair k-tiles
    # ktile k<4: heads (4k, 4k+2); ktile 4+k: heads (4k+1, 4k+3)
    woT_full = np.ascontiguousarray(np.asarray(Wo, np.float32).T)  # [d, e]
    order = []
    for hh in range(HPC):
        for k in range(NQ):
            order += [4 * k + hh, 4 * k + 2 + hh]
    woT_perm = np.concatenate(
        [woT_full[g * DK:(g + 1) * DK] for g in order], axis=0)
    woT = _sbuf_tiled(woT_perm).astype(BF16NP)
    bo2 = np.asarray(bo, np.float32).reshape(1, D)
    scale = np.float32(1.0 / np.sqrt(DK))
    n_pat = patterns.shape[0]
    mpat2 = np.ascontiguousarray(
        patterns.transpose(1, 0, 2).reshape(P, n_pat * SBLK)).astype(BF16NP)
    in_maps = []
    for c in range(N_CORES):
        cols = slice(c * EL, (c + 1) * EL)
        in_maps.append({
            "xT": xT,
            "wqT": _sbuf_tiled(
                np.asarray(Wq, np.float32).T[:, cols] * scale).astype(BF16NP),
            "wkT": _sbuf_tiled(
                np.asarray(Wk, np.float32).T[:, cols]).astype(BF16NP),
            "wvT": _sbuf_tiled(
                np.asarray(Wv, np.float32).T[:, cols]).astype(BF16NP),
            "woT": woT,
            "bo": bo2,
            "mpat": mpat2,
        })
    return in_maps


def _run(inputs, trace=False):
    blocks, patterns = _classify_mask(inputs["mask"])
    nc = _build_program(blocks, patterns.shape[0])
    in_maps = _prepare_inputs(inputs["x"], inputs["Wq"], inputs["Wk"],
                              inputs["Wv"], inputs["Wo"], inputs["bo"],
                              patterns)
    res = run_bass_kernel_spmd(nc, in_maps, list(range(N_CORES)),
                               trace=trace)
    full = np.empty((B, S, D), np.float32)
    for c in range(N_CORES):
        b, j = divmod(c, NQ)
        full[b, j * SBLK:(j + 1) * SBLK, :] = res.results[c]["out"]
    return full, res


def kernel(**inputs) -> np.ndarray:
    out, _ = _run(inputs, trace=False)
    return out


# revision 19
# speedup vs baseline: 1.2990x; 1.0498x over previous
"""Multi-head causal self-attention on 8 Trainium2 NeuronCores.

Sharding: heads are split 2-per-core (tensor parallel); every core computes
QKV projections for its 2 heads over the full batch (bf16 activations and
weights), runs attention, then two half-size 8-core AllToAll collectives
(bf16, one per head, the first overlapped with the second head's attention
sweep) redistribute per-head outputs to sequence-sharded form for the
row-parallel output projection.

Schedule: x streams in per 512-token column block; batch-0 QKV projection
runs under the DMA, batch-1 projection interleaves with batch-0 attention
sweeps, and the h0 half of the output projection runs under the second
AllToAll so the PE never waits on the collective tail.

Reference semantics (torch nn.Linear convention, y = x @ W.T):
  Q = x @ Wq.T ; K = x @ Wk.T ; V = x @ Wv.T           (split into 16 heads)
  scores = Q K^T / sqrt(64), causal-masked, softmax
  out = (softmax(scores) @ V, concat heads) @ Wo.T + bo
"""

import sys
from collections import deque
from contextlib import ExitStack

sys.path.insert(0, "/opt/trn_rl_repo")

import numpy as np
import ml_dtypes

import concourse.bass as bass  # noqa: E402
import concourse.mybir as mybir  # noqa: E402
from concourse import bacc  # noqa: E402
from concourse.bass_utils import run_bass_kernel_spmd  # noqa: E402
from concourse.masks import make_identity  # noqa: E402
from concourse.tile import TileContext  # noqa: E402

B = 2
S = 2048
D = 1024
H = 16
DK = 64
N_CORES = 8
HPC = H // N_CORES          # heads per core = 2
EL = HPC * DK               # local embedding slice = 128
P = 128                     # partitions
SBLK = 512                  # q-block (free dim of score matmuls)
NQ = S // SBLK              # q-blocks per batch = 4
NKT = S // P                # k-tiles per batch = 16
ND = D // P                 # d-tiles = 8
BS = B * S                  # flattened (batch, seq) = 4096
NCB = BS // SBLK            # 512-token column blocks = 8
F32 = mybir.dt.float32
F32R = mybir.dt.float32r
BF16 = mybir.dt.bfloat16
BF16NP = ml_dtypes.bfloat16


def _classify_mask(mask: np.ndarray):
    """Classify each (q-block, k-tile) block of the [S, S] mask.

    Returns (blocks, patterns):
      blocks[j] = list of (t, pat_idx or None) k-tiles with any valid entry
      patterns  = float32 [n_pat, P, SBLK] multiplicative masks in [k, q]
                  layout for partially-valid blocks (deduplicated).
    """
    mask = np.asarray(mask).astype(bool)
    patterns = []
    pat_index = {}
    blocks = []
    for j in range(NQ):
        row = []
        sub_q = mask[j * SBLK:(j + 1) * SBLK]
        for t in range(NKT):
            sub = sub_q[:, t * P:(t + 1) * P]
            if not sub.any():
                continue
            if sub.all():
                row.append((t, None))
                continue
            patT = np.ascontiguousarray(sub.T).astype(np.float32)  # [k, q]
            key = patT.tobytes()
            if key not in pat_index:
                pat_index[key] = len(patterns)
                patterns.append(patT)
            row.append((t, pat_index[key]))
        assert row, f"q-block {j} has no valid keys; unsupported mask"
        blocks.append(row)
    if not patterns:
        patterns.append(np.ones((P, SBLK), np.float32))
    return blocks, np.stack(patterns)


DEBUG_DUMP = False


def _build_program(blocks, n_pat):
    nc = bacc.Bacc("TRN2", target_bir_lowering=False, debug=False,
                   num_devices=N_CORES)

    # ---- I/O ----------------------------------------------------------
    # xT: [D, B*S] bf16 (x transposed, batches concatenated along columns)
    # weight inputs are pre-tiled on the host into SBUF layout [P, ND*cols]
    xT = nc.declare_dram_parameter("xT", [D, BS], BF16, isOutput=False)
    wqT = nc.declare_dram_parameter("wqT", [P, ND * EL], BF16,
                                    isOutput=False)
    wkT = nc.declare_dram_parameter("wkT", [P, ND * EL], BF16,
                                    isOutput=False)
    wvT = nc.declare_dram_parameter("wvT", [P, ND * EL], BF16,
                                    isOutput=False)
    woT = nc.declare_dram_parameter("woT", [P, ND * D], BF16,
                                    isOutput=False)
    bo = nc.declare_dram_parameter("bo", [1, D], F32, isOutput=False)
    mpat = nc.declare_dram_parameter("mpat", [P, n_pat * SBLK], BF16,
                                     isOutput=False)
    out = nc.declare_dram_parameter("out", [SBLK, D], F32, isOutput=True)

    # collective bounce buffers (internal DRAM), one pair per head
    a2a_in = [nc.dram_tensor(f"a2a_in{h}", [N_CORES, DK, SBLK], BF16)
              for h in range(HPC)]
    a2a_out = [nc.dram_tensor(f"a2a_out{h}", [N_CORES, DK, SBLK], BF16)
               for h in range(HPC)]
    dbg = {}
    if DEBUG_DUMP:
        for nm, shp in (("dbg_q", [P, BS]), ("dbg_k", [P, BS]),
                        ("dbg_va", [P, B * (S // P) * HPC * (DK + 1)]),
                        ("dbg_stage", [P, N_CORES * SBLK])):
            dbg[nm] = nc.declare_dram_parameter(nm, shp, BF16, isOutput=True)

    with TileContext(nc) as tc, ExitStack() as ctx:
        const = ctx.enter_context(tc.tile_pool(name="const", bufs=1))
        persist = ctx.enter_context(tc.tile_pool(name="persist", bufs=1))

        # constants
        ident = const.tile([P, P], F32)
        make_identity(nc, ident)
        bo_sb = const.tile([1, D], F32)
        nc.sync.dma_start(out=bo_sb[:], in_=bo[:])
        bo_bc = const.tile([P, D], F32)
        nc.gpsimd.partition_broadcast(bo_bc[:], bo_sb[:])
        # only wq is loaded up-front; wk/wv DMAs are emitted mid-way through
        # the first projection block so the first x block isn't queued
        # behind them
        w_src = {"wq": wqT, "wk": wkT, "wv": wvT}
        w_sb = {}
        for name in ("wq", "wk", "wv"):
            w_sb[name] = const.tile([P, ND * EL], BF16, name=f"w_{name}",
                                    tag=f"w_{name}")
        nc.sync.dma_start(out=w_sb["wq"][:], in_=wqT[:])
        # mpat / woT DMAs are emitted later (first use is phases B / D) so
        # their transfers don't delay the x stream at startup
        mpat_sb = const.tile([P, n_pat * SBLK], BF16, tag="mpat")
        zero_w = const.tile([P, DK + 1], BF16, tag="zero_w")
        nc.vector.memset(zero_w[:], 0.0)
        wrm = const.tile([P, SBLK], BF16, tag="wrm")
        nc.vector.memset(wrm[:], 0.0)
        woT_sb = const.tile([P, ND * D], BF16, tag="woT")

        # persistent activations
        qT = persist.tile([P, BS], BF16, tag="qT")   # [EL, B*S]
        kT = persist.tile([P, BS], BF16, tag="kT")
        # va: per 128-token tile t, cols [t*130, t*130+130) hold
        # [h0 v 64 | ones | h1 v 64 | ones]  (keys on partitions)
        VA_W = HPC * (DK + 1)
        va = persist.tile([P, B * NKT * VA_W], BF16, tag="va")
        va4 = va[:].rearrange("p (t h e) -> p t h e", t=B * NKT, h=HPC)
        nc.vector.memset(va4[:, :, :, DK:DK + 1], 1.0)
        # stage: rows [h*64,(h+1)*64) x cols [c*512,(c+1)*512) = normalized
        # head-h output for block c = b*NQ+j
        stage = persist.tile([P, N_CORES * SBLK], BF16, tag="stage")

        mpat3 = mpat_sb[:].rearrange("p (n q) -> p n q", n=n_pat)

        with ExitStack() as pmain:
            probs_pool = pmain.enter_context(tc.tile_pool(name="probs",
                                                          bufs=6))
            small = pmain.enter_context(tc.tile_pool(name="small", bufs=4))
            px = ExitStack()
            xpool = px.enter_context(tc.tile_pool(name="xT", bufs=2))
            vt_pool = px.enter_context(tc.tile_pool(name="vt", bufs=2))
            # PSUM budget: psA 2 + po 2 + ps_sc 2*2 = 8 banks
            psA = pmain.enter_context(
                tc.tile_pool(name="psA", bufs=2, space="PSUM"))
            po_pool = pmain.enter_context(
                tc.tile_pool(name="po", bufs=2, space="PSUM"))
            ps_sc = pmain.enter_context(
                tc.tile_pool(name="ps_sc", bufs=2, space="PSUM"))

            ones_w = const.tile([1, DK], F32R, tag="ones_w")
            nc.vector.memset(ones_w[:].bitcast(F32), 1.0)

            # ---- emission helpers -----------------------------------
            def proj_block(cb):
                """QKV projection for one 512-token column block."""
                xt = xpool.tile([P, ND * SBLK], BF16, tag="xt",
                                name=f"xt{cb}")
                # xt[p, k*512+c] = xT[k*128+p, cb*512+c]
                nc.sync.dma_start(
                    out=xt[:].rearrange("p (k c) -> p k c", k=ND),
                    in_=xT[:, cb * SBLK:(cb + 1) * SBLK]
                    .rearrange("(k p) c -> p k c", p=P))
                osl = bass.ds(cb * SBLK, SBLK)
                for name in ("wq", "wk"):
                    ps = psA.tile([P, SBLK], F32, tag="psA",
                                  name=f"ps_{name}_{cb}")
                    for k in range(ND):
                        nc.tensor.matmul(
                            ps[:], w_sb[name][:, bass.ts(k, EL)],
                            xt[:, bass.ts(k, SBLK)],
                            start=(k == 0), stop=(k == ND - 1))
                    dest = qT if name == "wq" else kT
                    nc.vector.tensor_copy(dest[:, osl], ps[:])
                    if cb == 0:
                        nxt = "wk" if name == "wq" else "wv"
                        nc.sync.dma_start(out=w_sb[nxt][:],
                                          in_=w_src[nxt][:])
                ps = psA.tile([P, SBLK], F32, tag="psA", name=f"ps_wv_{cb}")
                for k in range(ND):
                    nc.tensor.matmul(
                        ps[:], w_sb["wv"][:, bass.ts(k, EL)],
                        xt[:, bass.ts(k, SBLK)],
                        start=(k == 0), stop=(k == ND - 1))
                vt = vt_pool.tile([P, SBLK], F32R, tag="vt",
                                  name=f"vt{cb}")
                nc.vector.tensor_copy(vt[:], ps[:])
                # transpose V into va (keys on partitions)
                for i in range(SBLK // P):
                    tile_idx = cb * (SBLK // P) + i
                    pst = psA.tile([P, SBLK], F32, tag="psA",
                                   name=f"tr_{cb}_{i}")
                    nc.tensor.matmul(pst[:, 0:P],
                                     vt[:, bass.ts(i, P)].bitcast(F32),
                                     ident[:], is_transpose=True)
                    nc.vector.tensor_copy(
                        va4[:, tile_idx, :, 0:DK],
                        pst[:, 0:P].rearrange("p (h e) -> p h e", h=HPC))

            def emit_scores(it, pair):
                h, b, j = it["h"], it["b"], it["j"]
                hsl = bass.ds(h * DK, DK)
                q0 = b * S + j * SBLK
                w = len(pair) * SBLK
                ps = ps_sc.tile([P, 2 * SBLK], F32, tag="ps_sc")
                for i, (t, _pat) in enumerate(pair):
                    nc.tensor.matmul(
                        ps[:, bass.ts(i, SBLK)],
                        kT[hsl, bass.ds(b * S + t * P, P)],
                        qT[hsl, bass.ds(q0, SBLK)],
                        start=True, stop=True)
                pr = probs_pool.tile([P, 2 * SBLK], BF16, tag="probs")
                nc.scalar.activation(pr[:, 0:w], ps[:, 0:w],
                                     mybir.ActivationFunctionType.Exp)
                pats = [pat for _t, pat in pair]
                if (len(pats) == 2 and pats[0] is not None
                        and pats[1] == pats[0] + 1):
                    nc.vector.tensor_mul(
                        pr[:, 0:2 * SBLK], pr[:, 0:2 * SBLK],
                        mpat_sb[:, pats[0] * SBLK:(pats[0] + 2) * SBLK])
                else:
                    for i, (t, pat) in enumerate(pair):
                        if pat is not None:
                            nc.vector.tensor_mul(pr[:, bass.ts(i, SBLK)],
                                                 pr[:, bass.ts(i, SBLK)],
                                                 mpat3[:, pat])
                return pr

            def emit_attnv(it, pair, pr):
                h, b = it["h"], it["b"]
                if it["po"] is None:
                    it["po"] = po_pool.tile([P, SBLK], F32, tag="po",
                                            name=f"po_{h}_{b}_{it['j']}")
                po = it["po"]
                for i, (t, _pat) in enumerate(pair):
                    it["n_mm"] += 1
                    nc.tensor.matmul(
                        po[0:DK + 1, :],
                        va[:, bass.ds((b * NKT + t) * VA_W + h * (DK + 1),
                                      DK + 1)],
                        pr[:, bass.ts(i, SBLK)],
                        start=(it["n_mm"] == 1),
                        stop=(it["n_mm"] == it["total"]))
                if it["n_mm"] >= it["total"]:
                    emit_norm(it)

            def emit_norm(it):
                h, b, j = it["h"], it["b"], it["j"]
                po = it["po"]
                sumr = small.tile([1, SBLK], F32, tag="sumr")
                nc.vector.tensor_copy(sumr[:], po[DK:DK + 1, :])
                rc = small.tile([1, SBLK], F32, tag="rc")
                nc.vector.reciprocal_approx_fast(out=rc[:], in_=sumr[:])
                rb = small.tile([DK, SBLK], F32, tag="rb")
                nc.gpsimd.partition_broadcast(rb[:], rc[:])
                nc.vector.tensor_mul(
                    stage[h * DK:(h + 1) * DK,
                          bass.ts(b * NQ + j, SBLK)],
                    po[0:DK, :], rb[:])

            pend = deque()
            LAG = 4

            def attn_units(h, b):
                """One generator step per pair emission for sweep (h, b)."""
                for j in range(NQ):
                    tiles = blocks[j]
                    it = {"h": h, "b": b, "j": j, "po": None, "n_mm": 0,
                          "total": len(tiles)}
                    for c0 in range(0, len(tiles), 2):
                        pair = tiles[c0:c0 + 2]
                        pr = emit_scores(it, pair)
                        pend.append((it, pair, pr))
                        if len(pend) > LAG:
                            emit_attnv(*pend.popleft())
                        yield

            def drain():
                while pend:
                    emit_attnv(*pend.popleft())

            def stage_dma(h):
                # one trigger: stage rows of head h -> a2a_in[h][8,64,512]
                nc.sync.dma_start(
                    out=a2a_in[h][:].rearrange("c p q -> p c q"),
                    in_=stage[h * DK:(h + 1) * DK, :]
                    .rearrange("p (c q) -> p c q", c=N_CORES))

            # ---- master schedule ------------------------------------
            # warm the PE clock gate while the first DMAs land
            wps = psA.tile([P, SBLK], F32, tag="psA", name="warm_ps")
            for r in range(10):
                nc.tensor.matmul(wps[0:DK + 1, :], zero_w[:], wrm[:],
                                 start=True, stop=True)

            # phase A: batch-0 projection, pipelined with x DMA
            for cb in range(NCB // 2):
                proj_block(cb)
                if cb == 0:
                    nc.sync.dma_start(out=mpat_sb[:], in_=mpat[:])

            # phase B: batch-1 projection interleaved with batch-0 attention
            gens = [attn_units(0, 0), attn_units(1, 0)]

            def next_unit():
                while gens:
                    try:
                        next(gens[0])
                        return True
                    except StopIteration:
                        gens.pop(0)
                return False

            for cb in range(NCB // 2, NCB):
                proj_block(cb)
                for _ in range(10):
                    next_unit()
            while next_unit():
                pass
            drain()
            px.close()

            # phase C: batch-1 attention. Both stage DMAs are emitted before
            # either collective so neither DMA can inherit an ordering edge
            # on a collective's completion; A2A0 still fires as soon as
            # stage_dma(0)'s data lands, and overlaps the tail + phase-D h0
            # matmuls.
            for _ in attn_units(0, 1):
                pass
            drain()
            stage_dma(0)
            nc.sync.dma_start(out=woT_sb[:], in_=woT[:])

            for _ in attn_units(1, 1):
                pass
            drain()
            stage_dma(1)
            if DEBUG_DUMP:
                nc.sync.dma_start(out=dbg["dbg_q"][:], in_=qT[:])
                nc.sync.dma_start(out=dbg["dbg_k"][:], in_=kT[:])
                nc.sync.dma_start(out=dbg["dbg_va"][:], in_=va[:])
                nc.sync.dma_start(out=dbg["dbg_stage"][:], in_=stage[:])
            for h in range(HPC):
                nc.gpsimd.collective_compute(
                    "AllToAll", mybir.AluOpType.bypass,
                    replica_groups=[list(range(N_CORES))],
                    ins=[a2a_in[h][:]], outs=[a2a_out[h][:]])
            pmain.close()

            # phase D: output projection; h0 half runs under A2A1
            cat = [None, None]
            with ExitStack() as p3:
                cat_pool = p3.enter_context(tc.tile_pool(name="cat",
                                                         bufs=1))
                osb_pool = p3.enter_context(tc.tile_pool(name="osb",
                                                         bufs=3))

                def cat_dma(h):
                    cat[h] = cat_pool.tile([P, NQ * SBLK], BF16,
                                           name=f"cat{h}", tag=f"cat{h}")
                    src = a2a_out[h][:].rearrange("(k two) p q -> two p k q",
                                                  two=2)
                    for two in range(2):
                        nc.sync.dma_start(
                            out=cat[h][two * DK:(two + 1) * DK, :]
                            .rearrange("p (k q) -> p k q", k=NQ),
                            in_=src[two])

                cat_dma(0)
                with tc.tile_pool(name="ps_o", bufs=8,
                                  space="PSUM") as ps_o:
                    pss = {}
                    NEB = D // SBLK
                    for st in range(SBLK // P):
                        for eb in range(NEB):
                            pss[st, eb] = ps_o.tile(
                                [P, SBLK], F32, tag="ps_o",
                                name=f"ps_f_{st}_{eb}")
                    # h0 contraction tiles run while A2A1 is on the wire
                    for half in range(2):
                        if half == 1:
                            cat_dma(1)
                        for st in range(SBLK // P):
                            for eb in range(NEB):
                                for k in range(NQ):
                                    kk = half * NQ + k
                                    nc.tensor.matmul(
                                        pss[st, eb][:],
                                        cat[half][:, bass.ds(
                                            k * SBLK + st * P, P)],
                                        woT_sb[:, bass.ds(kk * D + eb * SBLK,
                                                          SBLK)],
                                        start=(kk == 0),
                                        stop=(kk == 2 * NQ - 1))
                    for st in range(SBLK // P):
                        for eb in range(NEB):
                            ot = osb_pool.tile([P, SBLK], F32, tag="osb")
                            nc.vector.tensor_add(
                                ot[:], pss[st, eb][:],
                                bo_bc[:, bass.ts(eb, SBLK)])
                            nc.sync.dma_start(
                                out=out[st * P:(st + 1) * P,
                                        eb * SBLK:(eb + 1) * SBLK],
                                in_=ot[:])

    nc.compile()
    return nc


def _sbuf_tiled(wT):
    # [D, E] -> [P, ND*E]: row p holds d-tiles k at columns [k*E, (k+1)*E)
    dd, e = wT.shape
    return np.ascontiguousarray(
        wT.reshape(dd // P, P, e).transpose(1, 0, 2).reshape(P, -1))


def _prepare_inputs(x, Wq, Wk, Wv, Wo, bo, patterns):
    x = np.asarray(x, np.float32)
    xT = np.ascontiguousarray(
        np.concatenate([x[b].T for b in range(B)], axis=1)).astype(BF16NP)
    # output projection: concat dims permuted into head-pair k-tiles
    # ktile k<4: heads (4k, 4k+2); ktile 4+k: heads (4k+1, 4k+3)
    woT_full = np.ascontiguousarray(np.asarray(Wo, np.float32).T)  # [d, e]
    order = []
    for hh in range(HPC):
        for k in range(NQ):
            order += [4 * k + hh, 4 * k + 2 + hh]
    woT_perm = np.concatenate(
        [woT_full[g * DK:(g + 1) * DK] for g in order], axis=0)
    woT = _sbuf_tiled(woT_perm).astype(BF16NP)
    bo2 = np.asarray(bo, np.float32).reshape(1, D)
    scale = np.float32(1.0 / np.sqrt(DK))
    n_pat = patterns.shape[0]
    mpat2 = np.ascontiguousarray(
        patterns.transpose(1, 0, 2).reshape(P, n_pat * SBLK)).astype(BF16NP)
    in_maps = []
    for c in range(N_CORES):
        cols = slice(c * EL, (c + 1) * EL)
        in_maps.append({
            "xT": xT,
            "wqT": _sbuf_tiled(
                np.asarray(Wq, np.float32).T[:, cols] * scale).astype(BF16NP),
            "wkT": _sbuf_tiled(
                np.asarray(Wk, np.float32).T[:, cols]).astype(BF16NP),
            "wvT": _sbuf_tiled(
                np.asarray(Wv, np.float32).T[:, cols]).astype(BF16NP),
            "woT": woT,
            "bo": bo2,
            "mpat": mpat2,
        })
    return in_maps


def _run(inputs, trace=False):
    blocks, patterns = _classify_mask(inputs["mask"])
    nc = _build_program(blocks, patterns.shape[0])
    in_maps = _prepare_inputs(inputs["x"], inputs["Wq"], inputs["Wk"],
                              inputs["Wv"], inputs["Wo"], inputs["bo"],
                              patterns)
    res = run_bass_kernel_spmd(nc, in_maps, list(range(N_CORES)),
                               trace=trace)
    full = np.empty((B, S, D), np.float32)
    for c in range(N_CORES):
        b, j = divmod(c, NQ)
        full[b, j * SBLK:(j + 1) * SBLK, :] = res.results[c]["out"]
    return full, res


def kernel(**inputs) -> np.ndarray:
    out, _ = _run(inputs, trace=False)
    return out
